# revision 1
# baseline (speedup 1.0000x reference)
"""APPNP (GNN message passing) on 8 Trainium2 NeuronCores via Bass.

Strategy (dst-partitioned, per sharding hint):
  - Nodes sharded 8 ways; core c owns dst nodes [c*Nc, (c+1)*Nc).
  - MLP encoder computed per-core on its node shard (weights replicated).
  - z table [N, 64] fp32 replicated in each core's DRAM; per PPR iteration:
      * edges (grouped by dst-block of 64, and by source chunk of <=25000 rows
        so gather indices fit int16) are gathered with SWDGE dma_gather
        (256B rows) in batches per (super-block of 4 dst-blocks x chunk),
      * messages = w * z[src] (DVE, fp32 -> bf16),
      * segment-sum via one-hot matmul: S01[e, d] = (dstloc[e] == d) built on
        DVE with a broadcast iota compare; PE accumulates S01.T @ msg into a
        PSUM tile per dst-block (tiles straddling block boundaries simply get
        one matmul per touched block; out-of-block rows compare to 0),
      * z_new = 0.9 * psum + 0.1 * h (DVE), DMA'd to the local z shard,
      * AllGather shards -> replicated z table.
  - Program structure (group sizes, tile straddling, matmul start/stop flags)
    is computed host-side from the max group size over all 8 cores, so the
    single SPMD program works for every core; per-core slack is padded with
    weight-0 slots.
"""

import math
import numpy as np
import ml_dtypes

import concourse.bass as bass
import concourse.mybir as mybir
import concourse.tile as tile
from concourse import bacc
from concourse.bass_utils import run_bass_kernel_spmd
from concourse.masks import make_identity


# Allow elem_size_bytes % 128 (payload) in dma_gather; the descriptor row
# stride (elem_step) still must be a 256B multiple, which the ISA requires.
import inspect as _inspect
import textwrap as _tw

def _patch_dma_gather():
    src = _inspect.getsource(bass.BassGpSimd.dma_gather)
    assert "% 256 == 0" in src
    src = src.replace("elem_size_bytes > 0 and elem_size_bytes % 256 == 0",
                      "elem_size_bytes > 0 and elem_size_bytes % 128 == 0")
    ns = vars(bass).copy()
    exec(_tw.dedent(src), ns)
    bass.BassGpSimd.dma_gather = ns["dma_gather"]

_patch_dma_gather()

F32 = mybir.dt.float32
BF16 = mybir.dt.float16
I16 = mybir.dt.int16

ALPHA = 0.1
NCORES = 8


class Cfg:
    def __init__(self, n_nodes=100000, in_f=500, hid=64, out_f=64, k_iters=10,
                 chunk_rows=25000, blk=32, sb_blocks=8):
        self.N = n_nodes
        self.IN_F = in_f
        self.HID = hid
        self.OUT_F = out_f
        self.K = k_iters
        self.CHROWS = chunk_rows
        self.BLK = blk          # dst nodes per block (psum granularity)
        self.SBB = sb_blocks    # blocks per super-block (gather batch)
        assert n_nodes % NCORES == 0
        self.Nc = n_nodes // NCORES
        self.NB = math.ceil(self.Nc / blk)            # blocks per core
        self.NSB = math.ceil(self.NB / sb_blocks)     # super-blocks per core
        self.CH = math.ceil(n_nodes / chunk_rows)     # source chunks
        assert chunk_rows <= 32768


# ---------------------------------------------------------------------------
# host-side edge preprocessing
# ---------------------------------------------------------------------------

class Structure:
    """Uniform (core-independent) program structure."""
    pass


def prep_edges(cfg, edge_src, edge_dst, edge_weight):
    """Returns (structure, per_core_arrays).

    structure: gather/matmul schedule shared by all cores.
    per-core arrays: eidx [128, NSLOT//16] i16, ew [128, NT] f32,
                     edl [128, NPT] bf16.
    """
    N, Nc, BLK, SBB, NB, NSB, CH, CHROWS = (cfg.N, cfg.Nc, cfg.BLK, cfg.SBB,
                                            cfg.NB, cfg.NSB, cfg.CH, cfg.CHROWS)
    core = edge_dst // Nc
    dstl = edge_dst - core * Nc          # dst local to core
    blk = dstl // BLK                    # block id 0..NB-1
    chunk = edge_src // CHROWS

    # group key: (core, blk, chunk) -> counts
    gkey = (core.astype(np.int64) * NB + blk) * CH + chunk
    counts = np.bincount(gkey, minlength=NCORES * NB * CH).reshape(NCORES, NB, CH)
    G = counts.max(axis=0)               # [NB, CH] uniform group sizes

    # stream layout per (sb, c): blocks of the sb in order, then pad to 128
    # slot offsets per (b, c) within the global slot stream
    slot_off = np.zeros((NB, CH), dtype=np.int64)
    sc_slot_base = np.zeros((NSB, CH), dtype=np.int64)   # stream start
    sc_nslots = np.zeros((NSB, CH), dtype=np.int64)
    pos = 0
    for sb in range(NSB):
        blo, bhi = sb * SBB, min((sb + 1) * SBB, NB)
        for c in range(CH):
            sc_slot_base[sb, c] = pos
            for b in range(blo, bhi):
                slot_off[b, c] = pos
                pos += int(G[b, c])
            pad = (-(pos - sc_slot_base[sb, c])) % 128
            pos += pad
            sc_nslots[sb, c] = pos - sc_slot_base[sb, c]
    NSLOT = pos
    NT = NSLOT // 128

    # pairs: per (sb, c), tiles x touched blocks
    st = Structure()
    st.sc = []            # per (sb,c): dict with gather + pair info
    pair_tile = []        # global tile index per pair
    pair_blk = []         # global block per pair
    first_pair_of_blk = {}
    last_pair_of_blk = {}
    npairs = 0
    for sb in range(NSB):
        blo, bhi = sb * SBB, min((sb + 1) * SBB, NB)
        for c in range(CH):
            base = int(sc_slot_base[sb, c])
            nsl = int(sc_nslots[sb, c])
            t0 = base // 128
            ntiles = nsl // 128
            pairs = []
            for tl in range(ntiles):
                s_lo = base + tl * 128
                s_hi = s_lo + 128
                for b in range(blo, bhi):
                    g_lo = int(slot_off[b, c])
                    g_hi = g_lo + int(G[b, c])
                    if g_lo < s_hi and g_hi > s_lo:   # intersect
                        pairs.append((tl, b - blo, npairs))
                        k = int(b)
                        if k not in first_pair_of_blk:
                            first_pair_of_blk[k] = npairs
                        last_pair_of_blk[k] = npairs
                        pair_tile.append(t0 + tl)
                        pair_blk.append(b)
                        npairs += 1
            st.sc.append(dict(sb=sb, c=c, slot_base=base, nslots=nsl,
                              t0=t0, ntiles=ntiles, p0=pairs[0][2] if pairs else npairs,
                              pairs=pairs))
    NPT = npairs
    st.NSLOT, st.NT, st.NPT = NSLOT, NT, NPT
    st.first_pair = first_pair_of_blk
    st.last_pair = last_pair_of_blk
    st.G = G

    # per-core slot-level arrays
    order = np.lexsort((edge_src, gkey))   # sort by group, then src (stable)
    e_sorted = order
    # slot position of each edge: groups are contiguous in sorted order
    gkey_s = gkey[e_sorted]
    # rank within group
    grp_start = np.zeros(NCORES * NB * CH + 1, dtype=np.int64)
    np.cumsum(np.bincount(gkey_s, minlength=NCORES * NB * CH), out=grp_start[1:])
    rank = np.arange(len(e_sorted)) - grp_start[gkey_s]
    core_s = core[e_sorted]
    blk_s = blk[e_sorted]
    chunk_s = chunk[e_sorted]
    slot = slot_off[blk_s, chunk_s] + rank     # slot within core's stream

    pair_tile = np.asarray(pair_tile, dtype=np.int64)
    pair_blk = np.asarray(pair_blk, dtype=np.int64)

    per_core = []
    for cid in range(NCORES):
        m = core_s == cid
        sl = slot[m]
        idx_slots = np.zeros(NSLOT, dtype=np.int16)
        w_slots = np.zeros(NSLOT, dtype=np.float32)
        dl_slots = np.full(NSLOT, -1000.0, dtype=np.float32)
        idx_slots[sl] = (edge_src[e_sorted][m] - chunk_s[m] * CHROWS).astype(np.int16)
        w_slots[sl] = edge_weight[e_sorted][m] / 16.0
        dl_slots[sl] = (dstl[e_sorted][m]).astype(np.float32)

        eidx = idx_slots.reshape(NT * 8, 16).T          # [16, NSLOT/16]
        eidx = np.tile(eidx, (8, 1))                    # replicate to 128 parts
        ew = w_slots.reshape(NT, 128).T                 # [128, NT]
        # dstloc per pair: relative to the pair's block
        dl_t = dl_slots.reshape(NT, 128)                # [tile, part]
        edl = dl_t[pair_tile] - (pair_blk[:, None] * BLK)   # [NPT, 128]
        edl = edl.T.astype(np.float16)          # [128, NPT]
        per_core.append(dict(eidx=np.ascontiguousarray(eidx),
                             ew=np.ascontiguousarray(ew),
                             edl=np.ascontiguousarray(edl)))
    return st, per_core


# ---------------------------------------------------------------------------
# bass program
# ---------------------------------------------------------------------------

def build_program(cfg, st, stage=3, single=False, nocc=False):
    nc = bacc.Bacc("TRN2", target_bir_lowering=False, debug=False,
                   num_devices=1 if single else NCORES, num_swdge_queues=4,
                   dynamic_dma_scratch_size=16384)
    N, Nc, BLK, SBB, NB, NSB, CH, CHROWS = (cfg.N, cfg.Nc, cfg.BLK, cfg.SBB,
                                            cfg.NB, cfg.NSB, cfg.CH, cfg.CHROWS)
    IN_F, HID, OUT_F, K = cfg.IN_F, cfg.HID, cfg.OUT_F, cfg.K
    NSLOT, NT, NPT = st.NSLOT, st.NT, st.NPT
    KC = math.ceil(IN_F / 125)  # k-chunks for mm1 (125 rows each, or less)

    # I/O
    x_h = nc.dram_tensor("x", [Nc, IN_F], F32, kind="ExternalInput")
    w1_h = nc.dram_tensor("W1", [IN_F, HID], F32, kind="ExternalInput")
    b1_h = nc.dram_tensor("b1", [1, HID], F32, kind="ExternalInput")
    w2_h = nc.dram_tensor("W2", [HID, OUT_F], F32, kind="ExternalInput")
    b2_h = nc.dram_tensor("b2", [1, OUT_F], F32, kind="ExternalInput")
    eidx_h = nc.dram_tensor("eidx", [128, NSLOT // 16], I16, kind="ExternalInput")
    ew_h = nc.dram_tensor("ew", [128, NT], F32, kind="ExternalInput")
    edl_h = nc.dram_tensor("edl", [128, NPT], BF16, kind="ExternalInput")
    zout_h = nc.dram_tensor("z_out", [Nc, OUT_F], F32, kind="ExternalOutput")

    # internal DRAM
    z_shard = nc.dram_tensor("z_shard", [Nc, 2 * OUT_F], BF16, kind="Internal")
    z_full = nc.dram_tensor("z_full", [N, 2 * OUT_F], BF16, kind="Internal",
                            addr_space="Shared")

    rg = [list(range(NCORES))]

    with tile.TileContext(nc) as tc:
        with tc.tile_pool(name="persist", bufs=1) as pp:
            # persistent SBUF
            eidx_sb = pp.tile([128, NSLOT // 16], I16)
            ew_sb = pp.tile([128, NT], F32)
            edl_sb = pp.tile([128, NPT], BF16)
            h01_sb = pp.tile([BLK, NB, OUT_F], F32)      # 0.1 * h
            iota_sb = pp.tile([128, 1, BLK], BF16)
            ident = pp.tile([128, 128], F32)
            ones_sb = pp.tile([1, 128], F32)
            w1_sb = pp.tile([125, KC, HID], F32)
            b1_sb = pp.tile([1, HID], F32)
            w2_sb = pp.tile([HID, OUT_F], F32)
            b2_sb = pp.tile([1, OUT_F], F32)

            nc.sync.dma_start(out=eidx_sb[:], in_=eidx_h.ap()[:])
            nc.sync.dma_start(out=ew_sb[:], in_=ew_h.ap()[:])
            nc.sync.dma_start(out=edl_sb[:], in_=edl_h.ap()[:])
            nc.sync.dma_start(out=b1_sb[:], in_=b1_h.ap()[:])
            nc.sync.dma_start(out=w2_sb[:], in_=w2_h.ap()[:])
            nc.sync.dma_start(out=b2_sb[:], in_=b2_h.ap()[:])
            for kc in range(KC):
                lo = kc * 125
                hi = min(lo + 125, IN_F)
                nc.sync.dma_start(out=w1_sb[: hi - lo, kc, :],
                                  in_=w1_h.ap()[lo:hi, :])
            make_identity(nc, ident[:])
            nc.gpsimd.memset(ones_sb[:], 1.0)
            iota16 = pp.tile([128, BLK], I16)
            nc.gpsimd.iota(iota16[:], pattern=[[1, BLK]], base=0,
                           channel_multiplier=0)
            nc.vector.tensor_copy(out=iota_sb[:, 0, :], in_=iota16[:])

            # ---------------- MLP encoder + z0 ----------------
            n_nt = math.ceil(Nc / 128)
            with tc.tile_pool(name="mlp_sb", bufs=3) as mp, \
                 tc.tile_pool(name="mlp_ps", bufs=2, space="PSUM") as mps:
                for it in range(n_nt):
                    r0 = it * 128
                    nt = min(128, Nc - r0)
                    x_sb = mp.tile([128, IN_F], F32, tag="x")
                    nc.sync.dma_start(out=x_sb[:nt], in_=x_h.ap()[r0:r0 + nt, :])
                    h1_ps = mps.tile([128, HID], F32, tag="h1")
                    for kc in range(KC):
                        lo = kc * 125
                        hi = min(lo + 125, IN_F)
                        xt_ps = mps.tile([125, 128], F32, tag="xt")
                        nc.tensor.transpose(out=xt_ps[: hi - lo, :nt],
                                            in_=x_sb[:nt, lo:hi],
                                            identity=ident[:nt, :nt])
                        xt_sb = mp.tile([125, 128], F32, tag="xts")
                        nc.vector.tensor_copy(out=xt_sb[: hi - lo, :nt],
                                              in_=xt_ps[: hi - lo, :nt])
                        nc.tensor.matmul(out=h1_ps[:nt], lhsT=xt_sb[: hi - lo, :nt],
                                         rhs=w1_sb[: hi - lo, kc, :],
                                         start=(kc == 0), stop=False)
                    nc.tensor.matmul(out=h1_ps[:nt], lhsT=ones_sb[:, :nt],
                                     rhs=b1_sb[:], start=False, stop=True)
                    a1_sb = mp.tile([128, HID], F32, tag="a1")
                    nc.scalar.activation(out=a1_sb[:nt], in_=h1_ps[:nt],
                                         func=mybir.ActivationFunctionType.Relu)
                    a1t_ps = mps.tile([HID, 128], F32, tag="a1t")
                    nc.tensor.transpose(out=a1t_ps[:, :nt], in_=a1_sb[:nt, :],
                                        identity=ident[:nt, :nt])
                    a1t_sb = mp.tile([HID, 128], F32, tag="a1ts")
                    nc.vector.tensor_copy(out=a1t_sb[:, :nt], in_=a1t_ps[:, :nt])
                    h_ps = mps.tile([128, OUT_F], F32, tag="h")
                    nc.tensor.matmul(out=h_ps[:nt], lhsT=a1t_sb[:, :nt],
                                     rhs=w2_sb[:], start=True, stop=False)
                    nc.tensor.matmul(out=h_ps[:nt], lhsT=ones_sb[:, :nt],
                                     rhs=b2_sb[:], start=False, stop=True)
                    z0_sb = mp.tile([128, OUT_F], BF16, tag="z0")
                    nc.scalar.activation(out=z0_sb[:nt], in_=h_ps[:nt],
                                         func=mybir.ActivationFunctionType.Copy)
                    nc.sync.dma_start(out=z_shard.ap()[r0:r0 + nt, :OUT_F],
                                      in_=z0_sb[:nt])
                    # h01 blocks (128/BLK sub-blocks per node tile)
                    for half in range(math.ceil(128 / BLK)):
                        lo = half * BLK
                        if lo >= nt:
                            break
                        rows = min(BLK, nt - lo)
                        bidx = (r0 + lo) // BLK
                        nc.scalar.activation(
                            out=h01_sb[:rows, bidx, :],
                            in_=h_ps[lo:lo + rows, :],
                            func=mybir.ActivationFunctionType.Copy,
                            scale=ALPHA)

            if single or nocc:
                nc.sync.dma_start(out=z_full.ap()[:Nc, :], in_=z_shard.ap()[:])
            else:
                nc.gpsimd.collective_compute(
                    "AllGather", mybir.AluOpType.bypass, replica_groups=rg,
                    ins=[z_shard.ap().opt()], outs=[z_full.ap().opt()])

            # ---------------- propagation iterations ----------------
            qrr = [0]
            with tc.tile_pool(name="gat", bufs=1) as gp, \
                 tc.tile_pool(name="gat2", bufs=1) as gp2, \
                 tc.tile_pool(name="spool", bufs=1) as sp, \
                 tc.tile_pool(name="zpool", bufs=2) as zp, \
                 tc.tile_pool(name="prop_ps", bufs=8, space="PSUM") as pps:
                for k in range(K):
                    for sb in range(NSB):
                        blo = sb * SBB
                        bhi = min(blo + SBB, NB)
                        ps_t = {}
                        for b in range(blo, bhi):
                            ps_t[b] = pps.tile([BLK, OUT_F], F32, tag="ps", name=f"ps_{b%8}")
                        h01k = zp.tile([BLK, bhi - blo, OUT_F], F32, tag="h01k",
                                       name="h01k")
                        nc.scalar.activation(
                            out=h01k[:], in_=h01_sb[:, blo:bhi, :],
                            func=mybir.ActivationFunctionType.Copy,
                            scale=float(16.0 ** (-(k + 1))))
                        if stage < 1:
                            continue
                        elist = [st.sc[sb * CH + c] for c in range(CH)]
                        ntl_tot = sum(e["ntiles"] for e in elist)
                        npr_tot = sum(len(e["pairs"]) for e in elist)
                        t0_sb = elist[0]["t0"]
                        p0_sb = elist[0]["p0"]
                        zg = gp.tile([128, ntl_tot, OUT_F], BF16, tag="zg")
                        toff = 0
                        for c, e in enumerate(elist):
                            nsl = e["nslots"]
                            if nsl == 0:
                                continue
                            rows = min(CHROWS, N - c * CHROWS)
                            for off in range(0, nsl, 1024):
                                nn = min(1024, nsl - off)
                                sb0 = e["slot_base"] + off
                                o = toff + off // 128
                                nc.gpsimd.dma_gather(
                                    out_ap=zg[:, o:o + nn // 128, :],
                                    in_ap=z_full.ap()[c * CHROWS:c * CHROWS + rows,
                                                      :OUT_F],
                                    idxs_ap=eidx_sb[:, sb0 // 16:(sb0 + nn) // 16],
                                    num_idxs=nn, num_idxs_reg=nn,
                                    elem_size=OUT_F, elem_step=2 * OUT_F,
                                    queue_num=qrr[0] % 4)
                                qrr[0] += 1
                            toff += e["ntiles"]
                        if stage < 2 or ntl_tot == 0:
                            continue
                        zgw = gp2.tile([128, ntl_tot, OUT_F], BF16, tag="zgw")
                        nc.vector.tensor_tensor(
                            out=zgw[:], in0=zg[:],
                            in1=ew_sb[:, t0_sb:t0_sb + ntl_tot].unsqueeze(2)
                                .to_broadcast([128, ntl_tot, OUT_F]),
                            op=mybir.AluOpType.mult)
                        if npr_tot == 0:
                            continue
                        s01 = sp.tile([128, npr_tot, BLK], BF16, tag="s01")
                        nc.vector.tensor_tensor(
                            out=s01[:],
                            in0=iota_sb[:].to_broadcast([128, npr_tot, BLK]),
                            in1=edl_sb[:, p0_sb:p0_sb + npr_tot].unsqueeze(2)
                                .to_broadcast([128, npr_tot, BLK]),
                            op=mybir.AluOpType.is_equal)
                        if stage < 3:
                            continue
                        toff = 0
                        for c, e in enumerate(elist):
                            for (tl, bl, pg) in e["pairs"]:
                                b = blo + bl
                                nc.tensor.matmul(
                                    out=ps_t[b][:], lhsT=s01[:, pg - p0_sb, :],
                                    rhs=zgw[:, toff + tl, :],
                                    start=(st.first_pair.get(b) == pg),
                                    stop=(st.last_pair.get(b) == pg))
                            toff += e["ntiles"]
                        for b in range(blo, bhi):
                            rows = min(BLK, Nc - b * BLK)
                            znew = zp.tile([BLK, OUT_F], BF16, tag="znew")
                            if stage >= 3 and b in st.first_pair:
                                nc.vector.scalar_tensor_tensor(
                                    out=znew[:rows], in0=ps_t[b][:rows],
                                    scalar=1.0 - ALPHA,
                                    op0=mybir.AluOpType.mult,
                                    in1=h01k[:rows, b - blo, :],
                                    op1=mybir.AluOpType.add)
                            else:
                                nc.vector.tensor_copy(out=znew[:rows],
                                                      in_=h01k[:rows, b - blo, :])
                            nc.sync.dma_start(
                                out=z_shard.ap()[b * BLK:b * BLK + rows, :OUT_F],
                                in_=znew[:rows])
                    if single or nocc:
                        nc.sync.dma_start(out=z_full.ap()[:Nc, :],
                                          in_=z_shard.ap()[:])
                    else:
                        nc.gpsimd.collective_compute(
                            "AllGather", mybir.AluOpType.bypass,
                            replica_groups=rg,
                            ins=[z_shard.ap().opt()], outs=[z_full.ap().opt()])

            with tc.tile_pool(name="fin", bufs=3) as fp:
                for it in range(math.ceil(Nc / 128)):
                    r0 = it * 128
                    nt = min(128, Nc - r0)
                    zf = fp.tile([128, OUT_F], BF16, tag="zf")
                    nc.sync.dma_start(out=zf[:nt],
                                      in_=z_shard.ap()[r0:r0 + nt, :OUT_F])
                    zf2 = fp.tile([128, OUT_F], F32, tag="zf2")
                    nc.scalar.activation(out=zf2[:nt], in_=zf[:nt],
                                         func=mybir.ActivationFunctionType.Copy,
                                         scale=float(16.0 ** K))
                    nc.sync.dma_start(out=zout_h.ap()[r0:r0 + nt, :], in_=zf2[:nt])

    nc.compile()
    return nc


# ---------------------------------------------------------------------------
# entry point
# ---------------------------------------------------------------------------

_cache = {}


def _run(cfg, x, edge_src, edge_dst, edge_weight, W1, b1, W2, b2, trace=False):
    st, per_core = prep_edges(cfg, edge_src.astype(np.int64),
                              edge_dst.astype(np.int64),
                              edge_weight.astype(np.float32))
    key = (cfg.N, cfg.IN_F, cfg.K, st.NSLOT, st.NPT,
           tuple(int(g) for g in st.G.flat[:64]))
    if key not in _cache:
        _cache[key] = build_program(cfg, st)
    nc = _cache[key]
    in_maps = []
    for c in range(NCORES):
        in_maps.append({
            "x": np.ascontiguousarray(x[c * cfg.Nc:(c + 1) * cfg.Nc]),
            "W1": W1, "b1": b1.reshape(1, -1),
            "W2": W2, "b2": b2.reshape(1, -1),
            "eidx": per_core[c]["eidx"],
            "ew": per_core[c]["ew"],
            "edl": per_core[c]["edl"],
        })
    res = run_bass_kernel_spmd(nc, in_maps, core_ids=list(range(NCORES)),
                               trace=trace)
    out = np.concatenate([res.results[c]["z_out"] for c in range(NCORES)],
                         axis=0)
    return out, res


def kernel(x, edge_src, edge_dst, edge_weight, W1, b1, W2, b2):
    cfg = Cfg(n_nodes=x.shape[0], in_f=x.shape[1], hid=W1.shape[1],
              out_f=W2.shape[1], k_iters=10)
    out, _ = _run(cfg, np.asarray(x), np.asarray(edge_src),
                  np.asarray(edge_dst), np.asarray(edge_weight),
                  np.asarray(W1), np.asarray(b1), np.asarray(W2),
                  np.asarray(b2))
    return out

